# revision 2
# baseline (speedup 1.0000x reference)
"""Trainium2 Bass kernel for AtomGCNLayer (ResGatedGraphConv + BatchNorm + ReLU + residual).

Strategy (8 NeuronCores, SPMD):
  - Host: sort edges by target node; partition the 500k nodes into 64-node
    "windows" and the windows contiguously across the 8 cores (target-parallel
    sharding => no cross-core reduction of node aggregates needed).
    Host also pre-gathers x[tgt], x[src] rows per edge (index prep only) and
    lays out everything feature-major so all device DMA is dense.
  - Device phase 1 (per core): for each 128-edge block, one bf16 matmul
    computes gate-arg g = Wk^T x_t + Wq^T x_s + We^T ea + (bk+bq) and
    v = Wv^T x_s + bv (edge-major [128,32] PSUM out).  ACT sigmoid + DVE mul
    form msg = sigmoid(g)*v.  A one-hot matrix OH[edge,node-in-window] built
    with a single DVE is_equal per group scatters messages via PE matmul
    (OH^T @ msg accumulated in PSUM per 64-node window), plus a fused skip
    matmul (x@Ws + bs + bias) per window.  BN partial stats via ones-matmul.
  - Host: reduce the 8 cores' BN partial stats -> scale/shift.
  - Device phase 2: h_norm*scale+shift, ReLU, +x residual.
"""

import math

import numpy as np
import ml_dtypes

from concourse import bacc, mybir
import concourse.tile as tile
from concourse.bass_utils import run_bass_kernel_spmd

BF16 = ml_dtypes.bfloat16

N = 500000
E = 5000000
D = 16
NC = 8
W = 64            # nodes per scatter window
PW = 1024         # windows per core
NODES_C = W * PW  # 65536 nodes per core
NPAD = NC * NODES_C
NWIN = NC * PW
NBANK = PW // 64  # PSUM agg banks per core (64 windows per bank)
BN_EPS = 1e-5

_nc_cache = {}


def _build_phase1(B):
    GB = 2 * B if 2 * B * 32 <= 512 else B   # blocks per group
    WPG = GB // B                            # windows per group
    ngroups = PW // WPG
    gpb = 64 // WPG                          # groups per agg bank
    S_c = PW * B * 128                       # slots per core

    bf = mybir.dt.bfloat16
    f32 = mybir.dt.float32
    nc = bacc.Bacc(None, target_bir_lowering=False, debug=True)
    INP = nc.dram_tensor("inp", [49, S_c], bf, kind="ExternalInput")
    TREL = nc.dram_tensor("trel", [ngroups, 128, GB], bf, kind="ExternalInput")
    IOTA = nc.dram_tensor("iota", [128, W, GB], bf, kind="ExternalInput")
    WGT = nc.dram_tensor("wgt", [49, 32], bf, kind="ExternalInput")
    XSK = nc.dram_tensor("xsk", [17, NODES_C], f32, kind="ExternalInput")
    WSB = nc.dram_tensor("wsb", [17, 16], f32, kind="ExternalInput")
    H = nc.dram_tensor("h", [NBANK, 128, 32, 16], f32, kind="ExternalOutput")
    STATS = nc.dram_tensor("stats", [1, 1024], f32, kind="ExternalOutput")

    SIG = mybir.ActivationFunctionType.Sigmoid
    MUL = mybir.AluOpType.mult
    EQ = mybir.AluOpType.is_equal

    with tile.TileContext(nc) as tc:
        with (
            tc.tile_pool(name="const", bufs=1) as cpool,
            tc.tile_pool(name="sbuf", bufs=3) as pool,
            tc.tile_pool(name="xskp", bufs=2) as xpool,
            tc.tile_pool(name="pm", bufs=2, space="PSUM") as pm,
            tc.tile_pool(name="pa", bufs=2, space="PSUM") as pa,
            tc.tile_pool(name="pst", bufs=1, space="PSUM") as pst,
        ):
            wt = cpool.tile([49, 32], bf)
            nc.sync.dma_start(wt[:], WGT[:])
            wsb = cpool.tile([17, 16], f32)
            nc.sync.dma_start(wsb[:], WSB[:])
            it = cpool.tile([128, W, GB], bf)
            nc.sync.dma_start(it[:], IOTA[:])
            ones = cpool.tile([128, 1], f32)
            nc.gpsimd.memset(ones[:], 1.0)

            ssum = pst.tile([1, 512], f32, space="PSUM", tag="ssum")
            ssq = pst.tile([1, 512], f32, space="PSUM", tag="ssq")

            for k in range(NBANK):
                agg = pa.tile([128, 32, 16], f32, space="PSUM", tag="agg")
                xsk = xpool.tile([17, 64, W], f32, tag="xsk")
                nc.sync.dma_start(xsk[:], XSK[:, k * 4096:(k + 1) * 4096])
                for gg in range(gpb):
                    g = k * gpb + gg
                    ic = pool.tile([49, GB, 128], bf, tag="ic")
                    nc.sync.dma_start(ic[:], INP[:, g * GB * 128:(g + 1) * GB * 128])
                    tct = pool.tile([128, GB], bf, tag="tct")
                    nc.sync.dma_start(tct[:], TREL[g])
                    mm = pm.tile([128, GB, 32], f32, space="PSUM", tag="mm")
                    for b in range(GB):
                        nc.tensor.matmul(mm[:, b, :], lhsT=ic[:, b, :], rhs=wt[:],
                                         start=True, stop=True)
                    sg = pool.tile([128, GB, 16], bf, tag="sg")
                    nc.scalar.activation(sg[:], mm[:, :, 0:16], func=SIG)
                    msg = pool.tile([128, GB, 16], bf, tag="msg")
                    nc.vector.tensor_tensor(msg[:], sg[:], mm[:, :, 16:32], op=MUL)
                    oh = pool.tile([128, W, GB], bf, tag="oh")
                    nc.vector.tensor_tensor(
                        oh[:],
                        tct[:].unsqueeze(1).to_broadcast([128, W, GB]),
                        it[:],
                        op=EQ,
                    )
                    for wi in range(WPG):
                        win = gg * WPG + wi
                        pos = 64 * (win % 2)
                        col = win // 2
                        out_ap = agg[pos:pos + 64, col, :]
                        for b in range(B):
                            blk = wi * B + b
                            nc.tensor.matmul(out_ap, lhsT=oh[:, :, blk],
                                             rhs=msg[:, blk, :],
                                             start=(b == 0), stop=False,
                                             tile_position=(0, pos))
                        nc.tensor.matmul(out_ap, lhsT=xsk[:, win, :], rhs=wsb[:],
                                         start=False, stop=True,
                                         tile_position=(0, pos))
                hsb = pool.tile([128, 32, 16], f32, tag="hsb")
                nc.vector.tensor_copy(hsb[:], agg[:])
                nc.sync.dma_start(H[k], hsb[:])
                hsq = pool.tile([128, 32, 16], f32, tag="hsq")
                nc.vector.tensor_tensor(hsq[:], hsb[:], hsb[:], op=MUL)
                nc.tensor.matmul(ssum[:], lhsT=ones[:], rhs=hsb[:],
                                 start=(k == 0), stop=(k == NBANK - 1),
                                 skip_group_check=True)
                nc.tensor.matmul(ssq[:], lhsT=ones[:], rhs=hsq[:],
                                 start=(k == 0), stop=(k == NBANK - 1),
                                 skip_group_check=True)
            stsb = pool.tile([1, 1024], f32, tag="stsb")
            nc.vector.tensor_copy(stsb[:, 0:512], ssum[:])
            nc.vector.tensor_copy(stsb[:, 512:1024], ssq[:])
            nc.sync.dma_start(STATS[:], stsb[:])
    nc.compile()
    return nc


def _build_phase2():
    f32 = mybir.dt.float32
    nc = bacc.Bacc(None, target_bir_lowering=False, debug=True)
    H = nc.dram_tensor("h", [NBANK, 128, 32, 16], f32, kind="ExternalInput")
    X = nc.dram_tensor("x", [NBANK, 128, 32, 16], f32, kind="ExternalInput")
    SCL = nc.dram_tensor("scl", [128, 16], f32, kind="ExternalInput")
    SFT = nc.dram_tensor("sft", [128, 16], f32, kind="ExternalInput")
    Y = nc.dram_tensor("y", [NBANK, 128, 32, 16], f32, kind="ExternalOutput")
    ADD = mybir.AluOpType.add
    MUL = mybir.AluOpType.mult
    MAX = mybir.AluOpType.max
    with tile.TileContext(nc) as tc:
        with (
            tc.tile_pool(name="const", bufs=1) as cpool,
            tc.tile_pool(name="sbuf", bufs=3) as pool,
        ):
            scl = cpool.tile([128, 16], f32)
            nc.sync.dma_start(scl[:], SCL[:])
            sft = cpool.tile([128, 16], f32)
            nc.sync.dma_start(sft[:], SFT[:])
            scl_b = scl[:].unsqueeze(1).to_broadcast([128, 32, 16])
            sft_b = sft[:].unsqueeze(1).to_broadcast([128, 32, 16])
            for k in range(NBANK):
                h = pool.tile([128, 32, 16], f32, tag="h")
                nc.sync.dma_start(h[:], H[k])
                xb = pool.tile([128, 32, 16], f32, tag="xb")
                nc.sync.dma_start(xb[:], X[k])
                t1 = pool.tile([128, 32, 16], f32, tag="t1")
                nc.vector.tensor_tensor(t1[:], h[:], scl_b, op=MUL)
                nc.vector.tensor_tensor(t1[:], t1[:], sft_b, op=ADD)
                nc.vector.tensor_scalar(t1[:], t1[:], 0.0, None, op0=MAX)
                nc.vector.tensor_tensor(t1[:], t1[:], xb[:], op=ADD)
                yb = pool.tile([128, 32, 16], f32, tag="yb")
                nc.vector.tensor_copy(yb[:], t1[:])
                nc.sync.dma_start(Y[k], yb[:])
    nc.compile()
    return nc


def host_prep(x, edge_index, edge_attr):
    """Build all per-core device arrays. Index math + layout only."""
    src = np.asarray(edge_index[0], dtype=np.int64)
    tgt = np.asarray(edge_index[1], dtype=np.int64)
    x = np.asarray(x, dtype=np.float32)
    ea = np.asarray(edge_attr, dtype=np.float32)

    perm = np.argsort(tgt, kind="stable")
    tgt_s = tgt[perm]
    src_s = src[perm]
    wid = tgt_s // W
    counts = np.bincount(wid, minlength=NWIN)
    B = max(1, int(math.ceil(counts.max() / 128)))
    S_w = 128 * B
    S = NWIN * S_w
    S_c = PW * S_w
    starts = np.zeros(NWIN + 1, np.int64)
    starts[1:] = np.cumsum(counts)
    slots = wid * S_w + (np.arange(E, dtype=np.int64) - starts[wid])

    GB = 2 * B if 2 * B * 32 <= 512 else B
    ngroups = PW // (GB // B)

    x16 = x.astype(BF16)
    pay = np.zeros((S, 48), BF16)
    pay[slots, 0:16] = x16[tgt_s]
    pay[slots, 16:32] = x16[src_s]
    pay[slots, 32:48] = ea[perm].astype(BF16)

    trel = np.full(S, -1.0, np.float32)
    trel[slots] = (tgt_s % W).astype(np.float32)
    trel16 = trel.astype(BF16)

    xpad = np.zeros((NPAD, D), np.float32)
    xpad[:N] = x
    mask = np.zeros(NPAD, np.float32)
    mask[:N] = 1.0

    iota = np.broadcast_to(
        np.repeat(np.arange(W, dtype=np.float32), GB).astype(BF16).reshape(1, W, GB),
        (128, W, GB)).copy()

    in_maps = []
    for c in range(NC):
        inp_c = np.empty((49, S_c), BF16)
        inp_c[0:48] = pay[c * S_c:(c + 1) * S_c].T
        inp_c[48] = BF16(1.0)
        trel_c = (trel16[c * S_c:(c + 1) * S_c]
                  .reshape(ngroups, GB, 128).transpose(0, 2, 1).copy())
        xsk_c = np.empty((17, NODES_C), np.float32)
        xsk_c[0:16] = xpad[c * NODES_C:(c + 1) * NODES_C].T
        xsk_c[16] = mask[c * NODES_C:(c + 1) * NODES_C]
        in_maps.append({
            "inp": inp_c, "trel": trel_c, "iota": iota,
            "xsk": xsk_c,
        })
    return B, in_maps, xpad


def weight_arrays(Wk, bk, Wq, bq, Wv, bv, We, Ws, bs, bias):
    wgt = np.zeros((49, 32), np.float32)
    wgt[0:16, 0:16] = Wk
    wgt[16:32, 0:16] = Wq
    wgt[32:48, 0:16] = We
    wgt[48, 0:16] = bk + bq
    wgt[16:32, 16:32] = Wv
    wgt[48, 16:32] = bv
    wsb = np.zeros((17, 16), np.float32)
    wsb[0:16] = Ws
    wsb[16] = bs + bias
    return wgt.astype(BF16), wsb


def x_tiled(xpad):
    # [NC, NBANK, 128, 32, 16]; node n in core c: n = k*4096 + col*128 + p
    xt = xpad.reshape(NC, NBANK, 32, 128, D).transpose(0, 1, 3, 2, 4).copy()
    return xt


def untile_y(y_t):
    # y_t: [NC, NBANK, 128, 32, 16] -> [NPAD, 16]
    return y_t.transpose(0, 1, 3, 2, 4).reshape(NPAD, D)


def kernel(**inputs):
    x = np.asarray(inputs["x"], np.float32)
    B, in_maps, xpad = host_prep(x, inputs["edge_index"], inputs["edge_attr"])
    wgt, wsb = weight_arrays(
        np.asarray(inputs["Wk"], np.float32), np.asarray(inputs["bk"], np.float32),
        np.asarray(inputs["Wq"], np.float32), np.asarray(inputs["bq"], np.float32),
        np.asarray(inputs["Wv"], np.float32), np.asarray(inputs["bv"], np.float32),
        np.asarray(inputs["We"], np.float32), np.asarray(inputs["Ws"], np.float32),
        np.asarray(inputs["bs"], np.float32), np.asarray(inputs["bias"], np.float32))
    for m in in_maps:
        m["wgt"] = wgt
        m["wsb"] = wsb

    if ("p1", B) not in _nc_cache:
        _nc_cache[("p1", B)] = _build_phase1(B)
    nc1 = _nc_cache[("p1", B)]
    res1 = run_bass_kernel_spmd(nc1, in_maps, list(range(NC)))

    # host BN stats reduction
    tot = np.zeros(2, np.float64)
    s_sum = np.zeros(D, np.float64)
    s_sq = np.zeros(D, np.float64)
    for c in range(NC):
        st = res1.results[c]["stats"].astype(np.float64).reshape(2, 32, D)
        s_sum += st[0].sum(axis=0)
        s_sq += st[1].sum(axis=0)
    mean = s_sum / N
    var = s_sq / N - mean ** 2
    gamma = np.asarray(inputs["gamma"], np.float32).astype(np.float64)
    beta = np.asarray(inputs["beta"], np.float32).astype(np.float64)
    scale = gamma / np.sqrt(var + BN_EPS)
    shift = beta - mean * scale
    scl = np.broadcast_to(scale.astype(np.float32), (128, D)).copy()
    sft = np.broadcast_to(shift.astype(np.float32), (128, D)).copy()

    xt = x_tiled(xpad)
    in_maps2 = [{
        "h": res1.results[c]["h"],
        "x": xt[c],
        "scl": scl,
        "sft": sft,
    } for c in range(NC)]
    if "p2" not in _nc_cache:
        _nc_cache["p2"] = _build_phase2()
    nc2 = _nc_cache["p2"]
    res2 = run_bass_kernel_spmd(nc2, in_maps2, list(range(NC)))

    y_t = np.stack([res2.results[c]["y"] for c in range(NC)])
    y = untile_y(y_t)[:N]
    return y.astype(np.float32)
